# revision 40
# baseline (speedup 1.0000x reference)
"""Trainium2 Bass kernel for nn_Block_74861279969699 (dense transformer block).

Sharding (8 cores): attention is head-sharded (2 of 16 heads per core, all
batches); proj/MLP are token-sharded (512 of 4096 tokens per core). One
AllToAll moves the attention output from head-sharding to token-sharding.

Host<->device traffic is minimized for the axon tunnel (~70 MB/s): x enters
token-sharded f16 with the key-padding mask folded in as extra rows (8 MB
total, one put per split) and is AllGathered + PE-transposed on device; z
leaves token-major f16 (8 MB total). The call runs as NSPLIT=4 pipelined
single-batch sub-calls: queued uploads pay no extra fixed cost, each split's
output streams back (full-duplex) under the remaining uplink, and only the
last split's 2 MB drains serially. All weights are device-resident across
calls, and the jitted SPMD executable is built once (no per-call retrace or
NEFF recompile).

All matmuls run in float32r (tf32-like) with fp32 PSUM accumulation.
LayerNorm1 is folded algebraically into the QKV matmul (scale/shift fixed up
via rank-1 matmuls and a broadcast multiply at PSUM evacuation); LayerNorm2
is materialized explicitly (only 512 tokens per core).
"""

import numpy as np

import concourse.bass as bass
import concourse.mybir as mybir
import concourse.tile as tile
from concourse import bacc

F32 = mybir.dt.float32
F32R = mybir.dt.float32r
F16 = mybir.dt.float16
AF = mybir.ActivationFunctionType
ALU = mybir.AluOpType

P = 128
NCORES = 8
B, N, DIM = 4, 1024, 1024
H, HD = 16, 64
HIDDEN = 4096
EPS = 1e-5
T = B * N                 # 4096 tokens
TC = T // NCORES          # 512 tokens per core
TT = T // 512             # 8 token tiles of 512
KC = DIM // P             # 8 dim chunks
MH = HIDDEN // P          # 32 hidden chunks
HPC = H // NCORES         # 2 heads per core
NEG_MASK = -60.0

_CACHE = {}


def _build(reps: int = 1, stages=frozenset({'qkv','vtrans','attn','proj','mlp'}), loop_n: int | None = None, nbatch: int = B):
    nc = bacc.Bacc("TRN2", target_bir_lowering=False, debug=False,
                   num_devices=NCORES)
    kb = nbatch                   # batches this kernel processes
    kt = kb * N                   # tokens
    ktc = kt // NCORES            # tokens per core
    ktt = kt // 512               # 512-token tiles

    # ---- DRAM I/O (f32r-typed tensors receive f32 bits; no conversion) ----
    # xh rows [0:ktc] = this core's x shard; rows [ktc:ktc+kb] = key-padding
    # mask bias rows (0.0 / NEG_MASK, exact in f16; identical on every core).
    xh_d = nc.dram_tensor("xh", [ktc + kb, DIM], F16, kind="ExternalInput")
    wqkvT_d = nc.dram_tensor("wqkvT", [KC, P, 3 * P], F32R, kind="ExternalInput").ap()
    srow_d = nc.dram_tensor("srow", [1, 3 * P], F32R, kind="ExternalInput").ap()
    crow_d = nc.dram_tensor("crow", [1, 3 * P], F32R, kind="ExternalInput").ap()
    rpbT_d = nc.dram_tensor("rpbT", [HPC, KC, P, N], F32, kind="ExternalInput").ap()
    projT_d = nc.dram_tensor("projT", [KC, P, DIM], F32R, kind="ExternalInput").ap()
    projb_d = nc.dram_tensor("projb", [1, DIM], F32R, kind="ExternalInput").ap()
    n2w_d = nc.dram_tensor("n2w", [KC, P], F32, kind="ExternalInput").ap()
    n2b_d = nc.dram_tensor("n2b", [KC, P], F32, kind="ExternalInput").ap()
    w1T_d = nc.dram_tensor("w1T", [MH, P, KC, P], F32R, kind="ExternalInput").ap()
    b1_d = nc.dram_tensor("b1", [MH, P], F32, kind="ExternalInput").ap()
    w2T_d = nc.dram_tensor("w2T", [KC, P, MH, P], F32R, kind="ExternalInput").ap()
    b2row_d = nc.dram_tensor("b2row", [1, DIM], F32R, kind="ExternalInput").ap()
    ident_d = nc.dram_tensor("ident", [P, P], F32R, kind="ExternalInput").ap()
    identh_d = nc.dram_tensor("identh", [P, P], F16, kind="ExternalInput").ap()
    onesc_d = nc.dram_tensor("onesc", [P, 1], F32R, kind="ExternalInput").ap()
    onesr_d = nc.dram_tensor("onesr", [1, 512], F32R, kind="ExternalInput").ap()

    z_d = nc.dram_tensor("z", [ktc, DIM], F16, kind="ExternalOutput").ap()

    # internal DRAM: gathered f16 x, plus AllToAll buffers (f32; bitcast ends)
    xgin = nc.dram_tensor("xgin", [ktc, DIM], F16)
    xg = nc.dram_tensor("xg", [kt, DIM], F16, addr_space="Shared")
    cc_in = nc.dram_tensor("cc_in", [NCORES, P, ktc], F32)
    cc_out = nc.dram_tensor("cc_out", [NCORES, P, ktc], F32)

    env = locals()
    env["stages"] = stages
    env["loop_n"] = loop_n
    with tile.TileContext(nc) as tc:
        if loop_n is not None:
            with tc.For_i(0, loop_n, 1):
                _emit(nc, tc, env)
        else:
            for _rep in range(reps):
                _emit(nc, tc, env)
    nc.compile()
    return nc


def _emit(nc, tc, d):
    xh_d, xgin, xg, wqkvT_d = d["xh_d"], d["xgin"], d["xg"], d["wqkvT_d"]
    srow_d, crow_d, rpbT_d = d["srow_d"], d["crow_d"], d["rpbT_d"]
    projT_d, projb_d, n2w_d, n2b_d = d["projT_d"], d["projb_d"], d["n2w_d"], d["n2b_d"]
    w1T_d, b1_d, w2T_d, b2row_d = d["w1T_d"], d["b1_d"], d["w2T_d"], d["b2row_d"]
    z_d, cc_in, cc_out = d["z_d"], d["cc_in"], d["cc_out"]
    ident_d, identh_d, onesc_d, onesr_d = \
        d["ident_d"], d["identh_d"], d["onesc_d"], d["onesr_d"]
    stages = d["stages"]
    kb, kt, ktc, ktt = d["kb"], d["kt"], d["ktc"], d["ktt"]
    tcp = ktc // P                # 128-token subtiles per core

    # gather the token-sharded f16 x into full [kt, DIM] (overlaps const DMAs)
    if 'qkv' in stages:
        nc.sync.dma_start(xgin[:], xh_d[0:ktc])
        if d["loop_n"] is not None:
            nc.sync.dma_start(xg[0:ktc], xgin[:])  # timing-only stand-in
        else:
            nc.gpsimd.collective_compute(
                "AllGather", ALU.bypass,
                ins=[xgin[:]], outs=[xg[:]],
                replica_groups=[list(range(NCORES))],
            )

    with (
        tc.tile_pool(name="consts", bufs=1) as consts,
        tc.tile_pool(name="persistB", bufs=1) as persistB,
        tc.tile_pool(name="rows", bufs=6) as rows,
        tc.tile_pool(name="bcast", bufs=4) as bcast,
    ):
        # ---- constants ----
        ones_col = consts.tile([P, 1], F32R)
        nc.sync.dma_start(ones_col[:], onesc_d)
        ones_row = consts.tile([1, 512], F32R)
        nc.sync.dma_start(ones_row[:], onesr_d)
        ident = consts.tile([P, P], F32R)
        nc.sync.dma_start(ident[:], ident_d)
        identh = consts.tile([P, P], F16)
        nc.sync.dma_start(identh[:], identh_d)
        eps_sb = consts.tile([1, 1], F32)
        nc.vector.memset(eps_sb[:], EPS)
        srow_sb = consts.tile([1, 3 * P], F32R)
        nc.sync.dma_start(srow_sb[:], srow_d)
        crow_sb = consts.tile([1, 3 * P], F32R)
        nc.sync.dma_start(crow_sb[:], crow_d)
        maskh = consts.tile([P, kb, KC], F16)
        nc.sync.dma_start(maskh[:], xh_d[ktc:ktc + kb, :]
                          .rearrange("b (c p) -> p b c", p=P))
        mask_sb = consts.tile([P, kb, KC], F32)
        nc.vector.tensor_copy(mask_sb[:], maskh[:])
        wqkv_sb = consts.tile([P, KC, 3 * P], F32R)
        nc.sync.dma_start(wqkv_sb[:], wqkvT_d.rearrange("k p m -> p k m"))

        # persistent across phases
        yt_sb = persistB.tile([P, KC, ktc], F32R)    # post-attention residual

        with tc.tile_pool(name="persistA", bufs=1) as persistA:
            o_sb = persistA.tile([P, kt], F32R)      # attention out (2 heads)
            q_sb = persistA.tile([P, kt], F32R)
            k_sb = persistA.tile([P, kt], F32R)
            v_sb = persistA.tile([P, kt], F32R)
            vtok = [persistA.tile([P, 2 * 65], F32R, name=f"vtok{ti}")
                    for ti in range(kt // P)]

            # ================= Phase A: LN1-folded QKV =================
            with (
                tc.tile_pool(name="xraw", bufs=2) as xraw,
                tc.tile_pool(name="xstream", bufs=2) as xstream,
                tc.tile_pool(name="sqpool", bufs=3) as sqpool,
                tc.tile_pool(name="statps", bufs=1, space="PSUM") as statps,
                tc.tile_pool(name="xtps", bufs=2, space="PSUM") as xtps,
                tc.tile_pool(name="qkvps", bufs=3, space="PSUM") as qkvps,
            ):
                for tt in range(ktt if 'qkv' in stages else 0):
                    raw = xraw.tile([P, 4, DIM], F16, name="raw")
                    nc.sync.dma_start(
                        raw[:], xg[tt * 512:(tt + 1) * 512, :]
                        .rearrange("(s p) d -> p s d", p=P))
                    xt = xstream.tile([P, KC, 512], F32R, name="xt")
                    for kc in range(KC):
                        tp = xtps.tile([P, 512], F16, name="xtp")
                        for st in range(4):
                            nc.tensor.transpose(
                                tp[:, st * P:(st + 1) * P],
                                raw[:, st, kc * P:(kc + 1) * P], identh[:])
                        nc.vector.tensor_copy(xt[:, kc], tp[:])

                    mu_ps = statps.tile([1, 512], F32, name="mu_ps")
                    ss_ps = statps.tile([1, 512], F32, name="ss_ps")
                    for kc in range(KC):
                        nc.tensor.matmul(mu_ps[:], ones_col[:], xt[:, kc],
                                         start=(kc == 0), stop=(kc == KC - 1))
                    for kc in range(KC):
                        sq = sqpool.tile([P, 512], F32R, name="sq")
                        nc.scalar.activation(sq[:], xt[:, kc], AF.Square)
                        nc.tensor.matmul(ss_ps[:], ones_col[:], sq[:],
                                         start=(kc == 0), stop=(kc == KC - 1))

                    # stats rows
                    mun_r = rows.tile([1, 512], F32R, tag="row", name="mun_r")   # -mu
                    nc.vector.tensor_scalar_mul(mun_r[:], mu_ps[:], -1.0 / DIM)
                    ess = rows.tile([1, 512], F32, tag="row", name="ess")
                    nc.vector.tensor_scalar_mul(ess[:], ss_ps[:], 1.0 / DIM)
                    mu2 = rows.tile([1, 512], F32, tag="row", name="mu2")
                    nc.vector.tensor_tensor(mu2[:], mun_r[:], mun_r[:], ALU.mult)
                    var = rows.tile([1, 512], F32, tag="row", name="var")
                    nc.vector.tensor_tensor(var[:], ess[:], mu2[:], ALU.subtract)
                    sd_r = rows.tile([1, 512], F32R, tag="row", name="sd_r")
                    nc.scalar.activation(sd_r[:], var[:], AF.Sqrt, bias=eps_sb[:])
                    rstd = rows.tile([1, 512], F32, tag="row", name="rstd")
                    nc.vector.reciprocal(rstd[:], sd_r[:])
                    rstdB = bcast.tile([P, 512], F32, tag="bc", name="rstdB")
                    nc.gpsimd.partition_broadcast(rstdB[:], rstd[:])

                    for mch, dst in enumerate((q_sb, k_sb, v_sb)):
                        ps = qkvps.tile([P, 512], F32, name="qkvps")
                        for kc in range(KC):
                            nc.tensor.matmul(
                                ps[:], wqkv_sb[:, kc, mch * P:(mch + 1) * P],
                                xt[:, kc], start=(kc == 0), stop=False)
                        nc.tensor.matmul(ps[:], srow_sb[:, mch * P:(mch + 1) * P],
                                         mun_r[:], start=False, stop=False)
                        nc.tensor.matmul(ps[:], crow_sb[:, mch * P:(mch + 1) * P],
                                         sd_r[:], start=False, stop=True)
                        nc.vector.tensor_tensor(
                            dst[:, tt * 512:(tt + 1) * 512], ps[:], rstdB[:],
                            ALU.mult)

            # ============ Phase A2: transpose v to token-major ============
            with tc.tile_pool(name="vtps", bufs=3, space="PSUM") as vtps:
                for ti in range(kt // P if 'vtrans' in stages else 0):
                    vt = vtok[ti]
                    for h in range(2):
                        tp = vtps.tile([P, 64], F32R, name="vtp")
                        nc.tensor.transpose(
                            tp[:], v_sb[h * 64:(h + 1) * 64, ti * P:(ti + 1) * P],
                            ident[h * 64:(h + 1) * 64, h * 64:(h + 1) * 64])
                        nc.vector.tensor_copy(vt[:, h * 65:h * 65 + 64], tp[:])
                    nc.vector.tensor_copy(vt[:, 64:65], ones_col[:])
                    nc.vector.tensor_copy(vt[:, 129:130], ones_col[:])

            # ================= Phase B: attention =================
            with (
                tc.tile_pool(name="rpbpool", bufs=1) as rpbpool,
                tc.tile_pool(name="spool", bufs=2) as spool,
                tc.tile_pool(name="ppool", bufs=3) as ppool,
                tc.tile_pool(name="scoreps", bufs=2, space="PSUM") as scoreps,
                tc.tile_pool(name="ops", bufs=2, space="PSUM") as ops_pool,
            ):
                for h in range(HPC if 'attn' in stages else 0):
                    rpb_sb = rpbpool.tile([P, KC, N], F32, name="rpb")
                    nc.sync.dma_start(rpb_sb[:],
                                      rpbT_d[h].rearrange("k p q -> p k q"))
                    hs = slice(h * 64, (h + 1) * 64)
                    vs = slice(h * 65, h * 65 + 65)
                    for b in range(kb):
                        t0 = b * N
                        o_ps = [ops_pool.tile([65, 512], F32, name=f"o_ps{qt}")
                                for qt in range(2)]
                        for kc in range(KC):
                            s_ps = scoreps.tile([P, N], F32, name="s_ps")
                            for qt in range(2):
                                nc.tensor.matmul(
                                    s_ps[:, qt * 512:(qt + 1) * 512],
                                    k_sb[hs, t0 + kc * P: t0 + (kc + 1) * P],
                                    q_sb[hs, t0 + qt * 512: t0 + (qt + 1) * 512],
                                    start=True, stop=True)
                            s1 = spool.tile([P, N], F32, name="s1")
                            nc.vector.tensor_tensor(s1[:], s_ps[:], rpb_sb[:, kc],
                                                    ALU.add)
                            p_sb = ppool.tile([P, N], F32R, name="p_sb")
                            nc.scalar.activation(p_sb[:], s1[:], AF.Exp,
                                                 bias=mask_sb[:, b, kc:kc+1])
                            for qt in range(2):
                                nc.tensor.matmul(
                                    o_ps[qt][:], vtok[b * KC + kc][:, vs],
                                    p_sb[:, qt * 512:(qt + 1) * 512],
                                    start=(kc == 0), stop=(kc == KC - 1))
                        for qt in range(2):
                            recip = rows.tile([1, 512], F32, tag="row", name="recip")
                            nc.vector.reciprocal(recip[:], o_ps[qt][64:65, :])
                            recipB = bcast.tile([P, 512], F32, tag="bc", name="recipB")[0:64]
                            nc.gpsimd.partition_broadcast(recipB[:], recip[:])
                            nc.vector.tensor_tensor(
                                o_sb[hs, t0 + qt * 512: t0 + (qt + 1) * 512],
                                o_ps[qt][0:64, :], recipB[:], ALU.mult)

            # ============== Phase C: AllToAll (inside persistA) ==============
            if 'proj' in stages:
                nc.sync.dma_start(
                    cc_in[:].rearrange("s p t -> p s t").bitcast(F32R),
                    o_sb[:].rearrange("p (s t) -> p s t", s=NCORES))
                if d["loop_n"] is not None:
                    nc.sync.dma_start(cc_out[:], cc_in[:])  # timing-only stand-in
                else:
                    nc.gpsimd.collective_compute(
                        "AllToAll", ALU.bypass,
                        ins=[cc_in[:]], outs=[cc_out[:]],
                        replica_groups=[list(range(NCORES))],
                    )

        # ================= Phase C2: proj =================
        with (
            tc.tile_pool(name="ccpool", bufs=1) as ccpool,
            tc.tile_pool(name="projpool", bufs=1) as projpool,
            tc.tile_pool(name="projps", bufs=3, space="PSUM") as projps,
            tc.tile_pool(name="xslps", bufs=2, space="PSUM") as xslps,
        ):
            if 'proj' in stages:
                cco_sb = ccpool.tile([P, NCORES, ktc], F32R)
                nc.sync.dma_start(cco_sb[:],
                                  cc_out[:].rearrange("s p t -> p s t").bitcast(F32R))
                projw_sb = projpool.tile([P, KC, DIM], F32R)
                nc.sync.dma_start(projw_sb[:], projT_d.rearrange("k p m -> p k m"))
                projb_sb = projpool.tile([1, DIM], F32R)
                nc.sync.dma_start(projb_sb[:], projb_d)
                # residual slice: transpose this core's own x shard (f16)
                rawl = ccpool.tile([P, tcp, DIM], F16)
                nc.sync.dma_start(rawl[:],
                                  xh_d[0:ktc].rearrange("(s p) d -> p s d", p=P))
                xsl_sb = ccpool.tile([P, KC, ktc], F32)
                for kc in range(KC):
                    tp = xslps.tile([P, ktc], F16, name="xslp")
                    for st in range(tcp):
                        nc.tensor.transpose(
                            tp[:, st * P:(st + 1) * P],
                            rawl[:, st, kc * P:(kc + 1) * P], identh[:])
                    nc.vector.tensor_copy(xsl_sb[:, kc], tp[:])

            for mch in range(KC if 'proj' in stages else 0):
                ps = projps.tile([P, ktc], F32, name="projps")
                for kc in range(KC):
                    nc.tensor.matmul(ps[:], projw_sb[:, kc, mch * P:(mch + 1) * P],
                                     cco_sb[:, kc], start=(kc == 0), stop=False)
                nc.tensor.matmul(ps[:], projb_sb[:, mch * P:(mch + 1) * P],
                                 ones_row[:, :ktc], start=False, stop=True)
                nc.vector.tensor_tensor(yt_sb[:, mch], ps[:],
                                        xsl_sb[:, mch], ALU.add)

        # ================= Phase D: LN2 + MLP =================
        with (
            tc.tile_pool(name="ln2pool", bufs=1) as ln2pool,
            tc.tile_pool(name="hpool", bufs=1) as hpool,
            tc.tile_pool(name="w1pool", bufs=3) as w1pool,
            tc.tile_pool(name="w2pool", bufs=2) as w2pool,
            tc.tile_pool(name="sq2pool", bufs=2) as sq2pool,
            tc.tile_pool(name="zpool", bufs=2) as zpool,
            tc.tile_pool(name="statps", bufs=1, space="PSUM") as statps,
            tc.tile_pool(name="mlpps", bufs=3, space="PSUM") as mlpps,
            tc.tile_pool(name="ztps", bufs=2, space="PSUM") as ztps,
        ):
            # LN2 stats
            mu_ps = statps.tile([1, ktc], F32, name="mu_ps")
            ss_ps = statps.tile([1, ktc], F32, name="ss_ps")
            MLPON = 'mlp' in stages
            for kc in range(KC if MLPON else 0):
                nc.tensor.matmul(mu_ps[:], ones_col[:], yt_sb[:, kc],
                                 start=(kc == 0), stop=(kc == KC - 1))
            for kc in range(KC if MLPON else 0):
                sq = sq2pool.tile([P, ktc], F32R, name="sq2")
                nc.scalar.activation(sq[:], yt_sb[:, kc], AF.Square)
                nc.tensor.matmul(ss_ps[:], ones_col[:], sq[:],
                                 start=(kc == 0), stop=(kc == KC - 1))
            if not MLPON:
                z0 = zpool.tile([P, tcp, DIM], F16, name="z0")
                nc.vector.memset(z0[:], 0.0)
                for st in range(tcp):
                    nc.sync.dma_start(z_d[st * P:(st + 1) * P, :], z0[:, st, :])
                return
            mu_r = rows.tile([1, ktc], F32, tag="row", name="mu2_r")
            nc.vector.tensor_scalar_mul(mu_r[:], mu_ps[:], 1.0 / DIM)
            ess = rows.tile([1, ktc], F32, tag="row", name="ess2")
            nc.vector.tensor_scalar_mul(ess[:], ss_ps[:], 1.0 / DIM)
            mu2 = rows.tile([1, ktc], F32, tag="row", name="mu22")
            nc.vector.tensor_tensor(mu2[:], mu_r[:], mu_r[:], ALU.mult)
            var = rows.tile([1, ktc], F32, tag="row", name="var2")
            nc.vector.tensor_tensor(var[:], ess[:], mu2[:], ALU.subtract)
            sd_r = rows.tile([1, ktc], F32, tag="row", name="sd2")
            nc.scalar.activation(sd_r[:], var[:], AF.Sqrt, bias=eps_sb[:])
            rstd = rows.tile([1, ktc], F32, tag="row", name="rstd2")
            nc.vector.reciprocal(rstd[:], sd_r[:])
            rstdB = bcast.tile([P, ktc], F32, tag="bc", name="rstd2B")
            nc.gpsimd.partition_broadcast(rstdB[:], rstd[:])
            muB = bcast.tile([P, ktc], F32, tag="bc", name="mu2B")
            nc.gpsimd.partition_broadcast(muB[:], mu_r[:])

            n2w_sb = ln2pool.tile([P, KC], F32)
            nc.sync.dma_start(n2w_sb[:], n2w_d.rearrange("k p -> p k"))
            n2b_sb = ln2pool.tile([P, KC], F32)
            nc.sync.dma_start(n2b_sb[:], n2b_d.rearrange("k p -> p k"))
            b1_sb = ln2pool.tile([P, MH], F32)
            nc.sync.dma_start(b1_sb[:], b1_d.rearrange("m p -> p m"))
            b2_sb = ln2pool.tile([1, DIM], F32R)
            nc.sync.dma_start(b2_sb[:], b2row_d)

            ln2_sb = ln2pool.tile([P, KC, ktc], F32R)
            for kc in range(KC):
                t1 = sq2pool.tile([P, ktc], F32, name="ln2t1")
                nc.vector.tensor_tensor(t1[:], yt_sb[:, kc].bitcast(F32), muB[:],
                                        ALU.subtract)
                nc.vector.tensor_tensor(t1[:], t1[:], rstdB[:], ALU.mult)
                nc.vector.tensor_scalar(ln2_sb[:, kc], t1[:],
                                        n2w_sb[:, kc:kc+1], n2b_sb[:, kc:kc+1],
                                        ALU.mult, ALU.add)

            # MLP1: H = gelu(ln2 @ w1.T + b1)
            h_sb = hpool.tile([P, MH, ktc], F32R)
            for mh in range(MH):
                w1m = w1pool.tile([P, KC, P], F32R, name="w1m")
                nc.sync.dma_start(w1m[:], w1T_d[mh])
                ps = mlpps.tile([P, ktc], F32, tag="mlp", name="mlp1ps")
                for kc in range(KC):
                    nc.tensor.matmul(ps[:], w1m[:, kc], ln2_sb[:, kc],
                                     start=(kc == 0), stop=(kc == KC - 1))
                nc.scalar.activation(h_sb[:, mh], ps[:], AF.Gelu,
                                     bias=b1_sb[:, mh:mh+1])

            # MLP2: z = H @ w2.T + b2 + yt, PE-transposed to token-major f16
            zt_sb = hpool.tile([P, tcp, DIM], F16)
            for dch in range(KC):
                w2m = w2pool.tile([P, MH, P], F32R, name="w2m")
                nc.sync.dma_start(w2m[:], w2T_d[dch])
                ps = mlpps.tile([P, ktc], F32, tag="mlp", name="mlp2ps")
                for kh in range(MH):
                    nc.tensor.matmul(ps[:], w2m[:, kh], h_sb[:, kh],
                                     start=(kh == 0), stop=False)
                nc.tensor.matmul(ps[:], b2_sb[:, dch * P:(dch + 1) * P],
                                 ones_row[:, :ktc], start=False, stop=True)
                z16 = zpool.tile([P, ktc], F16, name="z16")
                nc.vector.tensor_tensor(z16[:], ps[:],
                                        yt_sb[:, dch].bitcast(F32), ALU.add)
                tp = ztps.tile([P, tcp, P], F16, name="ztp")
                for st in range(tcp):
                    nc.tensor.transpose(tp[:, st, :],
                                        z16[:, st * P:(st + 1) * P], identh[:])
                nc.vector.tensor_copy(zt_sb[:, :, dch * P:(dch + 1) * P], tp[:])
            for st in range(tcp):
                nc.sync.dma_start(z_d[st * P:(st + 1) * P, :], zt_sb[:, st, :])


# Input names whose per-core values differ (global = concat over cores on
# axis 0); everything else is broadcast to all cores (replicated sharding).
_SHARDED = {"wqkvT", "srow", "crow", "rpbT", "xh", "z"}

# Inputs that only feed weight-derived tensors (cached on device across calls).
_WEIGHT_KEYS = ("rel_pos_bias", "qkv_w", "q_bias", "v_bias", "proj_w",
                "proj_b", "norm1_w", "norm1_b", "norm2_w", "norm2_b",
                "mlp_w1", "mlp_b1", "mlp_w2", "mlp_b2")


def _make_runner(nc):
    """Build the jitted SPMD executable ONCE (mirrors bass2jax.run_bass_via_pjrt
    but caches the jit so warm calls skip retrace + NEFF recompile)."""
    import jax
    from jax.sharding import Mesh, PartitionSpec, NamedSharding
    from jax.experimental.shard_map import shard_map
    from concourse import bass2jax

    bass2jax.install_neuronx_cc_hook()
    part_name = nc.partition_id_tensor.name if nc.partition_id_tensor else None
    in_names, out_names, out_avals = [], [], []
    for alloc in nc.m.functions[0].allocations:
        if not isinstance(alloc, mybir.MemoryLocationSet):
            continue
        name = alloc.memorylocations[0].name
        if alloc.kind == "ExternalInput":
            if name != part_name:
                in_names.append(name)
        elif alloc.kind == "ExternalOutput":
            out_names.append(name)
            out_avals.append(jax.core.ShapedArray(
                tuple(alloc.tensor_shape), mybir.dt.np(alloc.dtype)))
    bind_names = tuple(in_names + out_names
                       + ([part_name] if part_name else []))

    def _body(*args):
        operands = list(args)
        if part_name is not None:
            operands.append(bass2jax.partition_id_tensor())
        return tuple(bass2jax._bass_exec_p.bind(
            *operands, out_avals=tuple(out_avals), in_names=bind_names,
            out_names=tuple(out_names), lowering_input_output_aliases=(),
            sim_require_finite=True, sim_require_nnan=True, nc=nc))

    devices = jax.devices()[:NCORES]
    mesh = Mesh(np.asarray(devices), ("core",))
    in_specs = tuple(
        PartitionSpec("core") if n in _SHARDED else PartitionSpec()
        for n in in_names + out_names)
    fn = jax.jit(shard_map(_body, mesh=mesh, in_specs=in_specs,
                           out_specs=(PartitionSpec("core"),) * len(out_names),
                           check_rep=False), keep_unused=True)

    def put(arr, name):
        spec = (PartitionSpec("core") if name in _SHARDED
                else PartitionSpec())
        return jax.device_put(arr, NamedSharding(mesh, spec))

    ktc = next(a.tensor_shape[0] for a in nc.m.functions[0].allocations
               if isinstance(a, mybir.MemoryLocationSet)
               and a.memorylocations[0].name == "z")
    zeros = put(np.zeros((NCORES * ktc, DIM), np.float16), "z")
    return {"fn": fn, "put": put, "in_names": in_names, "zeros": zeros}


def _prep_weights(inputs):
    """Host-side weight transforms → device-resident arrays (done once per
    distinct weight set; keyed on array identity)."""
    f = np.float32
    scale = np.float32(HD ** -0.5)
    qkv = inputs["qkv_w"].astype(f)
    n1w = inputs["norm1_w"].astype(f)
    n1b = inputs["norm1_b"].astype(f)
    q_bias, v_bias = inputs["q_bias"], inputs["v_bias"]
    w1 = inputs["mlp_w1"].astype(f)

    wqkvT_g = np.empty((NCORES * KC, P, 3 * P), f)
    srow_g = np.empty((NCORES, 3 * P), f)
    crow_g = np.empty((NCORES, 3 * P), f)
    for c in range(NCORES):
        r0 = 2 * c * HD
        rows_q = qkv[r0:r0 + 2 * HD]
        rows_k = qkv[DIM + r0:DIM + r0 + 2 * HD]
        rows_v = qkv[2 * DIM + r0:2 * DIM + r0 + 2 * HD]
        Wp = np.concatenate([rows_q * scale, rows_k, rows_v], 0) * n1w[None, :]
        srow_g[c] = Wp.sum(1)
        Cq = (rows_q @ n1b + q_bias[r0:r0 + 2 * HD]) * scale
        crow_g[c] = np.concatenate([Cq, rows_k @ n1b,
                                    rows_v @ n1b + v_bias[r0:r0 + 2 * HD]])
        wqkvT_g[c * KC:(c + 1) * KC] = \
            np.ascontiguousarray(Wp.T).reshape(KC, P, 3 * P)

    rpbT_g = np.ascontiguousarray(
        inputs["rel_pos_bias"].astype(f).transpose(0, 2, 1)
    ).reshape(NCORES * HPC, KC, P, N)

    return {
        "wqkvT": wqkvT_g,
        "srow": srow_g.reshape(NCORES * 1, 3 * P),
        "crow": crow_g.reshape(NCORES * 1, 3 * P),
        "rpbT": rpbT_g,
        "projT": np.ascontiguousarray(
            inputs["proj_w"].astype(f).T).reshape(KC, P, DIM),
        "projb": inputs["proj_b"].astype(f).reshape(1, DIM),
        "n2w": inputs["norm2_w"].astype(f).reshape(KC, P),
        "n2b": inputs["norm2_b"].astype(f).reshape(KC, P),
        "w1T": np.ascontiguousarray(
            w1.reshape(MH, P, KC, P).transpose(0, 3, 2, 1)),
        "b1": inputs["mlp_b1"].astype(f).reshape(MH, P),
        "w2T": np.ascontiguousarray(
            inputs["mlp_w2"].astype(f).reshape(KC, P, MH, P)
            .transpose(0, 3, 2, 1)),
        "b2row": inputs["mlp_b2"].astype(f).reshape(1, DIM),
        "ident": np.eye(P, dtype=f),
        "identh": np.eye(P, dtype=np.float16),
        "onesc": np.ones((P, 1), f),
        "onesr": np.ones((1, 512), f),
    }


NSPLIT = 4                # pipelined batch-splits per call
HB = B // NSPLIT          # batches per split
HT = HB * N               # tokens per split


def kernel(**inputs) -> np.ndarray:
    if "run" not in _CACHE:
        import concurrent.futures
        _CACHE["nc"] = _build(nbatch=HB)
        _CACHE["run"] = _make_runner(_CACHE["nc"])
        _CACHE["ex"] = concurrent.futures.ThreadPoolExecutor(8)
    run = _CACHE["run"]
    ex = _CACHE["ex"]

    wkey = tuple(id(inputs[k]) for k in _WEIGHT_KEYS)
    if _CACHE.get("wkey") != wkey:
        host_w = _prep_weights(inputs)
        _CACHE["wdev"] = {k: run["put"](v, k) for k, v in host_w.items()}
        _CACHE["wkey"] = wkey
        _CACHE["wrefs"] = [inputs[k] for k in _WEIGHT_KEYS]  # pin ids

    xsrc = np.asarray(inputs["x"]).reshape(T, DIM)
    mask16 = np.where(np.asarray(inputs["attn_mask"]).astype(bool),
                      0.0, NEG_MASK).astype(np.float16)   # [B, N]
    KTC = HT // NCORES
    # per-core shard = [KTC x-rows | HB mask rows] (mask folded into xh to
    # avoid separate replicated puts, which cost ~70 ms each on the tunnel)
    xaug = [np.empty((NCORES, KTC + HB, DIM), np.float16) for _ in range(NSPLIT)]

    def build_shard(hc):
        h, c = hc
        r0 = h * HT + c * KTC
        xaug[h][c, :KTC] = xsrc[r0:r0 + KTC]
        xaug[h][c, KTC:] = mask16[h * HB:(h + 1) * HB]

    # two pipelined half-batch calls: upload of half 2 and execution overlap
    # the (full-duplex) fetch of half 1's output; half 2's host-side shard
    # build overlaps half 1's wire transfer
    feed = dict(_CACHE["wdev"])
    zs = []
    for h in range(NSPLIT):
        list(ex.map(build_shard, [(h, c) for c in range(NCORES)]))
        feed["xh"] = run["put"](
            xaug[h].reshape(NCORES * (KTC + HB), DIM), "xh")     # async
        args = [feed[n] for n in run["in_names"]] + [run["zeros"]]
        (z_g,) = run["fn"](*args)
        z_g.copy_to_host_async()
        zs.append(z_g)

    out = np.empty((T, DIM), np.float32)

    def fetch(hs):
        h, s = hs
        out[h * HT + s.index[0].start:h * HT + s.index[0].stop] = \
            np.asarray(s.data)

    list(ex.map(fetch, [(h, s) for h, z_g in enumerate(zs)
                        for s in z_g.addressable_shards]))
    return out.reshape(B, N, DIM)

